# revision 1
# baseline (speedup 1.0000x reference)
"""CLS-guided MoE kernel for 8 Trainium2 NeuronCores.

Strategy: pure data parallel over the batch (B=8192 -> 1024 per core).
All activations/weights are used on-chip in feature-major ("transposed")
layout so the tensor engine can contract over features; the host passes
transposed *views* of the inputs (pure layout permutation, no compute)
and transposes the per-core outputs back. BatchNorm batch statistics are
combined across cores with two small AllReduces.

Matmul data is bf16 (cast during DMA); the attention/softmax/gating chain
runs in fp32 (fp32r on the PE).
"""

import numpy as np

FEAT = 1024
H = 16
E = 7
HD = 64
B = 8192
NCORES = 8
BLOC = B // NCORES  # 1024
NB = BLOC
KT = 64  # k-tiles over the 8*FEAT gating contraction
MT = 8  # feature tiles per 1024
NCH = 2
CH = NB // NCH  # 512
NME = 56  # (m, e) pairs: m in 0..7 (head pairs), e in 0..6
EPS = 1e-5
GS = 1.702  # quickgelu scale
ASCALE = HD ** -0.5  # 0.125
WRE_KT = 16  # k-tiles per W_re strip load

_CACHE = {}


def _host_consts():
    # S[m]: maps a tmp tile's partition p (feature m*128+p) to its head h = 2m + p//64
    S = np.zeros((MT, 128, 16), np.float32)
    for m in range(MT):
        for p in range(128):
            S[m, p, 2 * m + p // 64] = 1.0
    # attn_exp rows r = e*16 + h.  U sums over e for fixed h; V broadcasts back.
    U = np.zeros((112, 16), np.float32)
    for r in range(112):
        U[r, r % 16] = 1.0
    V = np.ascontiguousarray(U.T)
    # G[m*7+e]: broadcast gate row e*16+2m+j to output partitions j*64+o
    G = np.zeros((NME, 112, 128), np.float32)
    for m in range(MT):
        for e in range(E):
            for p in range(128):
                G[m * 7 + e, e * 16 + 2 * m + p // 64, p] = 1.0
    return S, U, V, G


def _build(sim=False):
    import concourse.bass as bass
    import concourse.mybir as mybir
    import concourse.tile as tile
    from concourse import bacc

    dt = mybir.dt
    f32, bf16, f32r = dt.float32, dt.bfloat16, dt.float32r
    AF = mybir.ActivationFunctionType
    OP = mybir.AluOpType

    nc = bacc.Bacc("TRN2", target_bir_lowering=False, debug=True, num_devices=NCORES,
                   dynamic_dma_scratch_size=16384)

    # ---- kernel parameters (per core) ----
    xt = nc.declare_dram_parameter("xt", [8 * FEAT, NB], f32, isOutput=False)
    wret = nc.declare_dram_parameter("wret", [8 * FEAT, FEAT], f32, isOutput=False)
    wqt = nc.declare_dram_parameter("wqt", [FEAT, FEAT], f32, isOutput=False)
    wkt = nc.declare_dram_parameter("wkt", [FEAT, FEAT], f32, isOutput=False)
    wet = nc.declare_dram_parameter("wet", [H, E, HD, HD], f32, isOutput=False)  # (h,e,d,o)
    bre = nc.declare_dram_parameter("bre", [FEAT], f32, isOutput=False)
    gre = nc.declare_dram_parameter("gre", [FEAT], f32, isOutput=False)
    bbre = nc.declare_dram_parameter("bbre", [FEAT], f32, isOutput=False)
    bee = nc.declare_dram_parameter("bee", [H, E, HD], f32, isOutput=False)
    gee = nc.declare_dram_parameter("gee", [H, E, HD], f32, isOutput=False)
    bbe = nc.declare_dram_parameter("bbe", [H, E, HD], f32, isOutput=False)
    ssel = nc.declare_dram_parameter("ssel", [MT, 128, 16], f32, isOutput=False)
    usel = nc.declare_dram_parameter("usel", [112, 16], f32, isOutput=False)
    vsel = nc.declare_dram_parameter("vsel", [16, 112], f32, isOutput=False)
    gsel = nc.declare_dram_parameter("gsel", [NME, 112, 128], f32, isOutput=False)
    outt = nc.declare_dram_parameter("outt", [E * FEAT, NB], f32, isOutput=True)

    RG = [list(range(NCORES))]

    with tile.TileContext(nc) as tc:
        with (
            tc.tile_pool(name="const", bufs=1) as cons,
            tc.tile_pool(name="stream", bufs=2) as stream,
            tc.tile_pool(name="qrelp", bufs=1) as qrelp,
            tc.tile_pool(name="work", bufs=3) as work,
            tc.tile_pool(name="ps", bufs=3, space="PSUM") as psp,
            tc.tile_pool(name="psy", bufs=2, space="PSUM") as psyp,
            tc.tile_pool(name="ps_acc", bufs=1, space="PSUM") as ps_acc,
            tc.tile_pool(name="dram", bufs=1, space="DRAM") as dram,
        ):
            # ---------------- phase A: resident loads ----------------
            # issue order matters: SWDGE descriptor generation is serial on Q7.
            xtp = tc.tile_pool(name="xtp", bufs=1)
            xtp_pool = xtp.__enter__()
            clsp = tc.tile_pool(name="clsp", bufs=1)
            clsp_pool = clsp.__enter__()

            strip00 = stream.tile([128, WRE_KT, 128], bf16, tag="wstrip",
                                  name="wre_0_0", bufs=3)
            nc.gpsimd.dma_start(
                strip00[:],
                wret[0:WRE_KT * 128, 0:128].rearrange("(kt p) n -> p kt n", p=128))
            xts = []
            for i in range(8):
                pool_i = clsp_pool if i == 0 else xtp_pool
                xti = pool_i.tile([128, MT, NB], bf16, name=f"xts_{i}")
                src = xt[i * FEAT:(i + 1) * FEAT, :].rearrange("(kt p) b -> p kt b", p=128)
                nc.gpsimd.dma_start(xti[:], src)
                xts.append(xti)

            with nc.allow_non_contiguous_dma(reason="small param loads"):
                bre_p = cons.tile([128, MT], f32, name="bre_p")
                gre_p = cons.tile([128, MT], f32, name="gre_p")
                bbre_p = cons.tile([128, MT], f32, name="bbre_p")
                nc.sync.dma_start(bre_p[:], bre[:].rearrange("(m p) -> p m", p=128))
                nc.sync.dma_start(gre_p[:], gre[:].rearrange("(m p) -> p m", p=128))
                nc.sync.dma_start(bbre_p[:], bbre[:].rearrange("(m p) -> p m", p=128))

                bee_p = cons.tile([128, NME], f32, name="bee_p")
                gee_p = cons.tile([128, NME], f32, name="gee_p")
                bbe_p = cons.tile([128, NME], f32, name="bbe_p")
                for t, dst in ((bee, bee_p), (gee, gee_p), (bbe, bbe_p)):
                    src = t[:].rearrange("(hp two) e d -> two d hp e", two=2)
                    for j in range(2):
                        for hp in range(MT):
                            nc.sync.dma_start(
                                dst[j * 64:(j + 1) * 64, hp * E:(hp + 1) * E],
                                src[j][:, hp, :])

                ssel_sb = cons.tile([128, MT, 16], f32r, name="ssel_sb")
                nc.gpsimd.dma_start(ssel_sb[:], ssel[:].rearrange("m p h -> p m h"))
            usel_sb = cons.tile([112, 16], f32r, name="usel_sb")
            nc.gpsimd.dma_start(usel_sb[:], usel[:])
            vsel_sb = cons.tile([16, 112], f32r, name="vsel_sb")
            nc.gpsimd.dma_start(vsel_sb[:], vsel[:])

            bre_sig = cons.tile([128, MT], f32, name="bre_sig")
            nc.vector.tensor_scalar_mul(bre_sig[:], bre_p[:], GS)
            bes_sig = cons.tile([128, NME], f32, name="bes_sig")
            nc.vector.tensor_scalar_mul(bes_sig[:], bee_p[:], GS)

            # stats buffers
            qstats = cons.tile([128, MT, NCH, 6], f32, name="qstats")
            ystats = cons.tile([128, NME, NCH, 6], f32, name="ystats")

            # DRAM scratch
            yspill = dram.tile([NME, 128, NB], bf16, name="yspill")
            qspill = dram.tile([MT, 128, NB], bf16, name="qspill")
            cin1 = dram.tile([128, MT, 2], f32, name="cin1")
            cout1 = dram.tile([128, MT, 2], f32, name="cout1")
            cin2 = dram.tile([128, NME, 2], f32, name="cin2")
            cout2 = dram.tile([128, NME, 2], f32, name="cout2")

            attn_exp = cons.tile([112, NB], f32r, name="attn_exp")
            gates = cons.tile([112, NB], f32r, name="gates")

            # ------- phase C+B interleaved: gating matmul + experts -------
            NKS = KT // WRE_KT

            def emit_expert_unit(e, m, lw):
                # computes 1.702*quickgelu(We x + be); the extra 1.702 factor
                # cancels in the batch norm (stats are taken of the same
                # scaled quantity), so downstream math is unchanged.
                me = m * 7 + e
                ps = psyp.tile([128, NB], f32, tag="psy", name=f"psy_{me}")
                for c in range(NCH):
                    nc.tensor.matmul(
                        ps[:, c * CH:(c + 1) * CH],
                        lw[:, m, :],
                        xts[1 + e][:, m, c * CH:(c + 1) * CH],
                        start=True, stop=True,
                    )
                yact = work.tile([128, NB], bf16, tag="w16", name=f"ya_{me}")
                nc.scalar.activation(yact[:], ps[:], AF.Silu,
                                     bias=bes_sig[:, me:me + 1], scale=GS)
                for c in range(NCH):
                    nc.vector.bn_stats(ystats[:, me, c, :],
                                       yact[:, c * CH:(c + 1) * CH])
                nc.sync.dma_start(yspill[me, :, :], yact[:])

            def load_lwe(e):
                lw = stream.tile([128, MT, 128], bf16, tag="lwe", name=f"lwe_{e}")
                nc.vector.memset(lw[:], 0.0)
                with nc.allow_non_contiguous_dma(reason="expert weight pack"):
                    wsrc = wet[:].rearrange("(hp two) e d o -> two d hp e o", two=2)
                    nc.gpsimd.dma_start(lw[0:64, :, 0:64], wsrc[0][:, :, e, :])
                    nc.gpsimd.dma_start(lw[64:128, :, 64:128], wsrc[1][:, :, e, :])
                return lw

            for m in range(MT):
                # group m hosts expert e = m-1 (its xts arrived by now); the
                # units are spread through the strip loop to fill PE stalls.
                lw = load_lwe(m - 1) if 1 <= m <= E - 1 else None
                units = list(range(MT)) if lw is not None else []
                upg = (len(units) + NKS - 1) // NKS if units else 0
                pscs = []
                for c in range(NCH):
                    pg = psp.tile([128, CH], f32, tag="ps", name=f"psg_{m}_{c}")
                    pscs.append(pg)
                for ks in range(NKS):
                    if m == 0 and ks == 0:
                        strip = strip00
                    else:
                        strip = stream.tile([128, WRE_KT, 128], bf16, tag="wstrip",
                                            name=f"wre_{m}_{ks}", bufs=3)
                        nc.gpsimd.dma_start(
                            strip[:],
                            wret[ks * WRE_KT * 128:(ks + 1) * WRE_KT * 128,
                                 m * 128:(m + 1) * 128]
                            .rearrange("(kt p) n -> p kt n", p=128),
                        )
                    for c in range(NCH):
                        for k in range(WRE_KT):
                            kt_g = ks * WRE_KT + k
                            nc.tensor.matmul(
                                pscs[c][:],
                                strip[:, k, :],
                                xts[kt_g // MT][:, kt_g % MT, c * CH:(c + 1) * CH],
                                start=(ks == 0 and k == 0),
                                stop=(ks == NKS - 1 and k == WRE_KT - 1),
                            )
                        for m2 in units[ks * upg + c:(ks + 1) * upg:NCH]:
                            emit_expert_unit(m - 1, m2, lw)
                # gating gelu tail: qg = 1.702*quickgelu(W_re q_in + b_re)
                qg = work.tile([128, NB], bf16, tag="w16", name=f"qg_{m}")
                for c in range(NCH):
                    nc.scalar.activation(qg[:, c * CH:(c + 1) * CH], pscs[c][:],
                                         AF.Silu, bias=bre_sig[:, m:m + 1], scale=GS)
                for c in range(NCH):
                    nc.vector.bn_stats(qstats[:, m, c, :],
                                       qg[:, c * CH:(c + 1) * CH])
                nc.sync.dma_start(qspill[m, :, :], qg[:])
            # remaining expert: e = 6 hosted after the last gating group
            lw = load_lwe(E - 1)
            for m2 in range(MT):
                emit_expert_unit(E - 1, m2, lw)

            # cls tensor no longer needed
            clsp.__exit__(None, None, None)

            # ---------------- collective 1: query BN stats ----------------
            qmv = cons.tile([128, MT, 2], f32, name="qmv")
            for m in range(MT):
                nc.vector.bn_aggr(qmv[:, m, :], qstats[:, m, :, :])
            pay1 = cons.tile([128, MT, 2], f32, name="pay1")
            nc.vector.tensor_scalar_mul(pay1[:, :, 0], qmv[:, :, 0], 1.0 / NCORES)
            t1 = work.tile([128, MT], f32, tag="small", name="t1")
            nc.vector.tensor_tensor(t1[:], qmv[:, :, 0], qmv[:, :, 0], OP.mult)
            nc.vector.tensor_tensor(t1[:], t1[:], qmv[:, :, 1], OP.add)
            nc.vector.tensor_scalar_mul(pay1[:, :, 1], t1[:], 1.0 / NCORES)
            nc.sync.dma_start(cin1[:], pay1[:])
            if sim:
                nc.sync.dma_start(cout1[:], cin1[:])
            else:
                nc.gpsimd.collective_compute(
                    "AllReduce", OP.add, replica_groups=RG,
                    ins=[cin1.opt()], outs=[cout1.opt()])
            gs1 = cons.tile([128, MT, 2], f32, name="gs1")
            nc.sync.dma_start(gs1[:], cout1[:])

            # ---------------- collective 2: expert BN stats ----------------
            ymv = cons.tile([128, NME, 2], f32, name="ymv")
            for me in range(NME):
                nc.vector.bn_aggr(ymv[:, me, :], ystats[:, me, :, :])
            pay2 = cons.tile([128, NME, 2], f32, name="pay2")
            nc.vector.tensor_scalar_mul(pay2[:, :, 0], ymv[:, :, 0], 1.0 / NCORES)
            t2 = work.tile([128, NME], f32, tag="small", name="t2")
            nc.vector.tensor_tensor(t2[:], ymv[:, :, 0], ymv[:, :, 0], OP.mult)
            nc.vector.tensor_tensor(t2[:], t2[:], ymv[:, :, 1], OP.add)
            nc.vector.tensor_scalar_mul(pay2[:, :, 1], t2[:], 1.0 / NCORES)
            nc.sync.dma_start(cin2[:], pay2[:])
            if sim:
                nc.sync.dma_start(cout2[:], cin2[:])
            else:
                nc.gpsimd.collective_compute(
                    "AllReduce", OP.add, replica_groups=RG,
                    ins=[cin2.opt()], outs=[cout2.opt()])
            gs2 = cons.tile([128, NME, 2], f32, name="gs2")
            nc.sync.dma_start(gs2[:], cout2[:])

            # ---------------- BN scale/shift coefficients ----------------
            def bn_coeffs(gsx, g_p, b_p, n, s_out, t_out):
                var = work.tile([128, n], f32, tag="small", name=f"var{n}")
                nc.vector.tensor_tensor(var[:], gsx[:, :, 0], gsx[:, :, 0], OP.mult)
                nc.vector.tensor_tensor(var[:], gsx[:, :, 1], var[:], OP.subtract)
                nc.vector.tensor_scalar_add(var[:], var[:], EPS)
                sd = work.tile([128, n], f32, tag="small", name=f"sd{n}")
                nc.scalar.activation(sd[:], var[:], AF.Sqrt)
                inv = work.tile([128, n], f32, tag="small", name=f"inv{n}")
                nc.vector.reciprocal(inv[:], sd[:])
                nc.vector.tensor_tensor(s_out[:], inv[:], g_p[:], OP.mult)
                tmp = work.tile([128, n], f32, tag="small", name=f"tmp{n}")
                nc.vector.tensor_tensor(tmp[:], gsx[:, :, 0], s_out[:], OP.mult)
                nc.vector.tensor_tensor(t_out[:], b_p[:], tmp[:], OP.subtract)

            qs = cons.tile([128, MT], f32, name="qs")
            qt_sh = cons.tile([128, MT], f32, name="qt_sh")
            bn_coeffs(gs1, gre_p, bbre_p, MT, qs, qt_sh)

            es = cons.tile([128, NME], f32, name="es")
            et = cons.tile([128, NME], f32, name="et")
            bn_coeffs(gs2, gee_p, bbe_p, NME, es, et)

            # ---- back half, chunk-outer so H(c) overlaps E/F(c+1) ----
            wk_sb = cons.tile([128, MT, FEAT], bf16, name="wk_sb")
            nc.gpsimd.dma_start(wk_sb[:], wkt[:].rearrange("(kt p) n -> p kt n", p=128))
            hp_ctx = tc.tile_pool(name="hp", bufs=6)
            hp = hp_ctx.__enter__()

            def emit_h_unit(me, c):
                e, m = me % 7, me // 7
                gst = hp.tile([112, 128], f32r, tag="gsel_t",
                              name=f"gst_{me}_{c}", bufs=2)
                nc.gpsimd.dma_start(gst[:], gsel[me])
                ps = psp.tile([128, CH], f32, tag="ps", name=f"psg2_{me}_{c}")
                nc.tensor.matmul(
                    ps[:], gst[:],
                    gates[:, c * CH:(c + 1) * CH],
                    start=True, stop=True)
                yrel = hp.tile([128, CH], bf16, tag="yrel",
                               name=f"yr_{me}_{c}", bufs=4)
                nc.scalar.dma_start(yrel[:], yspill[me, :, c * CH:(c + 1) * CH])
                ybn = hp.tile([128, CH], f32, tag="ybn",
                              name=f"ybn_{me}_{c}", bufs=2)
                nc.scalar.activation(
                    ybn[:], yrel[:], AF.Identity,
                    bias=et[:, me:me + 1], scale=es[:, me:me + 1])
                ofin = hp.tile([128, CH], f32, tag="ofin",
                               name=f"of_{me}_{c}", bufs=3)
                nc.vector.tensor_tensor(ofin[:], ybn[:], ps[:], OP.mult)
                nc.sync.dma_start(
                    outt[e * FEAT + m * 128: e * FEAT + (m + 1) * 128,
                         c * CH:(c + 1) * CH],
                    ofin[:])

            for c in range(NCH):
                # ---- E(c): reload gelu'd query, BN-apply, q projection ----
                qrel = qrelp.tile([128, MT, CH], bf16, tag="qrel", name=f"qrel_{c}")
                nc.sync.dma_start(
                    qrel[:],
                    qspill[:, :, c * CH:(c + 1) * CH].rearrange("m p b -> p m b"))
                qT = qrelp.tile([128, MT, CH], bf16, tag="qT", name=f"qT_{c}")
                for m in range(MT):
                    nc.vector.tensor_scalar(
                        qrel[:, m, :], qrel[:, m, :],
                        qs[:, m:m + 1], qt_sh[:, m:m + 1], OP.mult, OP.add)
                for m in range(MT):
                    wq_s = stream.tile([128, MT, 128], bf16, tag="wstrip",
                                       name=f"wq_{c}_{m}", bufs=3)
                    nc.gpsimd.dma_start(
                        wq_s[:],
                        wqt[:, m * 128:(m + 1) * 128]
                        .rearrange("(kt p) n -> p kt n", p=128),
                    )
                    ps = psp.tile([128, CH], f32, tag="ps", name=f"psq_{c}_{m}")
                    for k in range(MT):
                        nc.tensor.matmul(
                            ps[:], wq_s[:, k, :], qrel[:, k, :],
                            start=(k == 0), stop=(k == MT - 1))
                    nc.any.tensor_copy(qT[:, m, :], ps[:])

                # ---- F(c): k projection + attention logits ----
                # (H units of the previous chunk are interleaved here)
                h_units = list(range(NME)) if c > 0 else []
                hi = 0
                for e in range(E):
                    pa = ps_acc.tile([16, CH], f32, tag="at", name=f"at_{e}_{c}")
                    for m in range(MT):
                        ps = psp.tile([128, CH], f32, tag="ps", name=f"psk_{e}_{m}_{c}")
                        for k in range(MT):
                            nc.tensor.matmul(
                                ps[:], wk_sb[:, k, m * 128:(m + 1) * 128],
                                xts[1 + e][:, k, c * CH:(c + 1) * CH],
                                start=(k == 0), stop=(k == MT - 1))
                        tmp = work.tile([128, CH], f32r, tag="w32",
                                        name=f"qk_{e}_{m}_{c}", bufs=2)
                        nc.vector.tensor_tensor(
                            tmp[:], ps[:], qT[:, m, :], OP.mult)
                        nc.tensor.matmul(
                            pa[:],
                            ssel_sb[:, m, :],
                            tmp[:],
                            start=(m == 0), stop=(m == MT - 1))
                        if hi < len(h_units):
                            emit_h_unit(h_units[hi], c - 1)
                            hi += 1
                    ex16 = work.tile([16, CH], f32r, tag="ex16",
                                     name=f"ex_{e}_{c}", bufs=2)
                    nc.scalar.activation(ex16[:], pa[:], AF.Exp, scale=ASCALE)
                    nc.sync.dma_start(
                        attn_exp[e * 16:(e + 1) * 16, c * CH:(c + 1) * CH], ex16[:])
                while hi < len(h_units):
                    emit_h_unit(h_units[hi], c - 1)
                    hi += 1

                # ---- G(c): softmax over experts -> gates ----
                ps_s = psp.tile([128, CH], f32, tag="ps", name=f"pss_{c}")
                nc.tensor.matmul(
                    ps_s[0:16, :], usel_sb[:],
                    attn_exp[:, c * CH:(c + 1) * CH],
                    start=True, stop=True)
                rs = work.tile([16, CH], f32r, tag="w32", name=f"rs_{c}", bufs=2)
                with nc.allow_low_precision(reason="softmax denom in fp32r"):
                    nc.vector.reciprocal(rs[:], ps_s[0:16, :])
                ps_b = psp.tile([128, CH], f32, tag="ps", name=f"psb_{c}")
                nc.tensor.matmul(
                    ps_b[0:112, :], vsel_sb[:], rs[:],
                    start=True, stop=True)
                nc.vector.tensor_tensor(
                    gates[:, c * CH:(c + 1) * CH],
                    attn_exp[:, c * CH:(c + 1) * CH], ps_b[0:112, :], OP.mult)

            # ---- tail: H units of the last chunk ----
            for me in range(NME):
                emit_h_unit(me, NCH - 1)

            hp_ctx.__exit__(None, None, None)
            xtp.__exit__(None, None, None)

    nc.compile()
    return nc


def _get_nc():
    if "nc" not in _CACHE:
        _CACHE["nc"] = _build()
    return _CACHE["nc"]


def _in_maps(inputs):
    S, U, V, G = _host_consts()
    f32 = np.float32
    cls_token = np.asarray(inputs["cls_token"], f32)
    xs = [np.asarray(inputs[f"x{i+1}"], f32) for i in range(7)]
    shared = {
        "wret": np.ascontiguousarray(np.asarray(inputs["W_re"], f32).T),
        "wqt": np.ascontiguousarray(np.asarray(inputs["Wq"], f32).T),
        "wkt": np.ascontiguousarray(np.asarray(inputs["Wk"], f32).T),
        "wet": np.ascontiguousarray(np.asarray(inputs["We"], f32).transpose(0, 1, 3, 2)),
        "bre": np.asarray(inputs["b_re"], f32),
        "gre": np.asarray(inputs["g_re"], f32),
        "bbre": np.asarray(inputs["bb_re"], f32),
        "bee": np.asarray(inputs["be"], f32),
        "gee": np.asarray(inputs["g_e"], f32),
        "bbe": np.asarray(inputs["bb_e"], f32),
        "ssel": S, "usel": U, "vsel": V, "gsel": G,
    }
    in_maps = []
    for i in range(NCORES):
        sl = slice(i * BLOC, (i + 1) * BLOC)
        xt_i = np.ascontiguousarray(
            np.concatenate([cls_token[sl].T] + [x[sl].T for x in xs], axis=0))
        m = {"xt": xt_i}
        m.update(shared)
        in_maps.append(m)
    return in_maps


def kernel(**inputs):
    from concourse.bass_utils import run_bass_kernel_spmd

    nc = _get_nc()
    res = run_bass_kernel_spmd(nc, _in_maps(inputs), core_ids=list(range(NCORES)))
    out = np.empty((B, E * FEAT), np.float32)
    for i in range(NCORES):
        out[i * BLOC:(i + 1) * BLOC] = res.results[i]["outt"].T
    return out

